# revision 17
# baseline (speedup 1.0000x reference)
"""Trainium2 Bass kernel for nn_CABlock (ClassAttention block + GroupConvMlp).

Contract: kernel(**inputs) takes FULL unsharded inputs (as produced by
reference.setup_inputs()) and returns the FULL output tuple
(x_cls_out [64,1,768], x_img [64,784,768]).  x_img is an identity
passthrough in the reference, so it is returned host-side and never
touches the device.

Sharding: data-parallel over batch B=64 across 8 NeuronCores (8 batches
per core, all parameters replicated, no collectives).

Math folds done host-side (all exact):
  - LN1 gamma/beta folded into Wq/Wk/Wv (+ biases).
  - w1_b dropped entirely (additive per-row constant pre-softmax is
    shift-invariant under softmax over the key axis).
  - pre-softmax head mixing (I + w1_W) and the 1/sqrt(hd) scale folded
    into a per-batch block-diagonal Q matrix built as
    qT[d] * repeat(SCALE*(I+w1_W), 24, axis=0)[d, g].
  - post-softmax mixing (I + w2_W) plus w2_b handled by appending a
    ones-row to the softmaxed attention (its V-matmul row is colsum(V),
    which pairs with the w2_b row of the [33,768] expanded mixing mask),
    then an elementwise mask multiply and a mask-matmul partition
    reduction that also scatters batch b's o-vector into row b.
  - LN2 gamma/beta folded into fc1 weights/bias.
"""

import sys

try:
    import concourse.bass as bass  # noqa: F401
except ImportError:
    sys.path.insert(0, "/opt/trn_rl_repo")

import numpy as np

import concourse.bass as bass
import concourse.tile as tile
from concourse import mybir
from concourse.bass_utils import run_bass_kernel_spmd

# ----------------------------------------------------------------------------
# Workaround for walrus "Too many sync wait commands" on the TileContext
# kernel-tail drain: split the accumulated sem-waits across multiple SP
# drain instructions (one wait each).
# ----------------------------------------------------------------------------
import concourse.tile as _tile_mod
from concourse.vector_clock import ScopedClock as _ScopedClock


def _patched_drain_and_barrier(self, tick_clock, wait_clock):
    nc = self.nc
    drain_inst = nc.sync.drain()
    wait_clock.add_sem_waits(
        drain_inst.ins, _ScopedClock({None: tick_clock.global_clock})
    )
    si = drain_inst.ins.sync_info
    waits = list(si.on_wait or []) if si is not None else []
    if len(waits) > 1:
        si.on_wait = waits[:1]
        for w in waits[1:]:
            extra = nc.sync.drain()
            if extra.ins.sync_info is None:
                extra.ins.sync_info = mybir.SyncInfo(on_wait=[w], on_update=[])
            else:
                extra.ins.sync_info.on_wait = [w]

    nc.all_engine_barrier()
    assert self.sems is not None
    popped = nc._tile_sem_poison_stack.pop()
    assert popped is self._sem_poison
    nc.clear_and_free_semaphores(list(self.sems.allocated().values()))
    nc.all_engine_barrier()


_tile_mod.TileContext._drain_and_barrier = _patched_drain_and_barrier

# Same walrus limitation applies to every instruction encoding (observed on
# Drain/CTRL and Matmult/S3_LW): at most MAX_WAITS sem-waits fit on one
# instruction.  Split the excess onto preceding same-engine ENGINE_NOPs.
MAX_WAITS = 1


def split_excess_waits(nc, limit=MAX_WAITS):
    for f in nc.m.functions:
        for bb in f.blocks:
            old = list(bb.instructions)
            out = []
            for inst in old:
                si = getattr(inst, "sync_info", None)
                waits = list(si.on_wait) if (si and si.on_wait) else []
                if len(waits) > limit:
                    si.on_wait = waits[:limit]
                    rest = waits[limit:]
                    eng = nc.engines[inst.engine]
                    for i in range(0, len(rest), limit):
                        if inst.engine != mybir.EngineType.DVE:
                            nop = mybir.InstDrain(
                                name=nc.get_next_instruction_name(), ins=[], outs=[]
                            )
                            nop.engine = inst.engine
                        else:
                            nop = eng._isa(
                                nc.isa.Opcode.NEURON_ISA_TPB_OPCODE_ENGINE_NOP, {}
                            )
                        nop.sync_info = mybir.SyncInfo(
                            on_wait=rest[i : i + limit], on_update=[]
                        )
                        nc.register_instruction(nop, overwrite=True)
                        out.append(nop)
                out.append(inst)
            bb.instructions[:] = out

# ---------------------------------------------------------------------------
# Problem constants (hardcoded per spec)
# ---------------------------------------------------------------------------
B, N, D = 64, 784, 768
H, HD, G = 32, 24, 2
HID = 3072
SCALE = HD ** -0.5
EPS = 1e-6
N_CORES = 8
BL = B // N_CORES          # batches per core = 8
M = N + 1                  # 785 rows per batch
DT = D // 128              # 6 d-tiles
MT = (M + 127) // 128      # 7 row tiles per batch (last has 17 rows)
M_LAST = M - (MT - 1) * 128  # 17
M2 = M + 1                   # fp32r needs even moving-dim; pad transposed space
ML2 = M2 - (MT - 1) * 128    # 18 rows in last padded tile
CG = D // G                # 384  per-group input channels
HG = HID // G              # 1536 per-group hidden channels

F32 = mybir.dt.float32
F32R = mybir.dt.float32r
AF = mybir.ActivationFunctionType
ALU = mybir.AluOpType
AX = mybir.AxisListType


def r(ap):
    """bitcast an AP to float32r for full-rate PE streaming."""
    return ap.bitcast(F32R)


class _PhaseDone(Exception):
    pass


def build_program(flags):
    """Emit the single-core SPMD Bass program. flags: dict of bias-presence."""
    import os
    PHASE = int(os.environ.get("KPHASE", "4"))
    nc = bass.Bass(trn_type="TRN2")

    u = nc.dram_tensor("u", [BL * M, D], F32, kind="ExternalInput")
    wq = nc.dram_tensor("wq", [D, D], F32R, kind="ExternalInput")
    wk = nc.dram_tensor("wk", [D, D], F32R, kind="ExternalInput")
    wv = nc.dram_tensor("wv", [D, D], F32R, kind="ExternalInput")
    wp = nc.dram_tensor("wp", [D, D], F32R, kind="ExternalInput")
    w1e = nc.dram_tensor("w1e", [D, H], F32, kind="ExternalInput")
    w2e = nc.dram_tensor("w2e", [H + 1, D], F32, kind="ExternalInput")
    fc1t = nc.dram_tensor("fc1t", [G, CG, HG], F32R, kind="ExternalInput")
    fc2t = nc.dram_tensor("fc2t", [G, HG, CG], F32R, kind="ExternalInput")
    ident = nc.dram_tensor("ident", [128, 128], F32, kind="ExternalInput")
    mask8d = nc.dram_tensor("mask8d", [H + 1, BL * BL], F32R, kind="ExternalInput")
    need_ones = any(flags[k] for k in ("bq", "bv", "bp", "b1", "b2"))
    if need_ones:
        onesd = nc.dram_tensor("onesd", [1, 128], F32R, kind="ExternalInput")
    if flags["bk"]:
        bkt = nc.dram_tensor("bkt", [DT, 128], F32, kind="ExternalInput")
    if flags["bq"]:
        bqr = nc.dram_tensor("bqr", [1, D], F32R, kind="ExternalInput")
    if flags["bv"]:
        bvr = nc.dram_tensor("bvr", [1, D], F32R, kind="ExternalInput")
    if flags["bp"]:
        bpr = nc.dram_tensor("bpr", [1, D], F32R, kind="ExternalInput")
    if flags["b1"]:
        b1r = nc.dram_tensor("b1r", [G, HG], F32R, kind="ExternalInput")
    if flags["b2"]:
        b2r = nc.dram_tensor("b2r", [G, CG], F32R, kind="ExternalInput")
    xo = nc.dram_tensor("xo", [BL, D], F32, kind="ExternalOutput")

    u3 = u.ap().rearrange("(b m) d -> b m d", b=BL)

    with tile.TileContext(nc) as tc:
        import contextlib

        try:
          with contextlib.ExitStack() as ctx:
            consts = ctx.enter_context(tc.tile_pool(name="consts", bufs=1))
            big = ctx.enter_context(tc.tile_pool(name="big", bufs=1))
            workA = ctx.enter_context(tc.tile_pool(name="workA", bufs=2))
            workB = ctx.enter_context(tc.tile_pool(name="workB", bufs=1))
            xpool = ctx.enter_context(tc.tile_pool(name="xpool", bufs=3))
            wppool = ctx.enter_context(tc.tile_pool(name="wppool", bufs=3))
            fc1pool = ctx.enter_context(tc.tile_pool(name="fc1pool", bufs=4))
            fc2pool = ctx.enter_context(tc.tile_pool(name="fc2pool", bufs=6))
            psB = ctx.enter_context(tc.tile_pool(name="psB", bufs=3, space="PSUM"))
            psT = ctx.enter_context(tc.tile_pool(name="psT", bufs=2, space="PSUM"))

            # --- constants / weights resident in SBUF ---
            id_sb = consts.tile([128, 128], F32, tag="ident")
            nc.sync.dma_start(out=id_sb, in_=ident.ap())

            def load_w(name, dram):
                t = consts.tile([128, DT * D], F32R, tag=name)
                for i in range(DT):
                    nc.sync.dma_start(
                        out=t[:, i * D : (i + 1) * D],
                        in_=dram.ap()[i * 128 : (i + 1) * 128, :],
                    )
                return t

            wq_sb = load_w("wq", wq)
            wk_sb = load_w("wk", wk)
            wv_sb = load_w("wv", wv)

            w1_sb = consts.tile([128, DT * H], F32, tag="w1e")
            for i in range(DT):
                nc.sync.dma_start(
                    out=w1_sb[:, i * H : (i + 1) * H],
                    in_=w1e.ap()[i * 128 : (i + 1) * 128, :],
                )
            w2_sb = consts.tile([H + 1, D], F32, tag="w2e")
            nc.sync.dma_start(out=w2_sb, in_=w2e.ap())

            # per-batch ones-column masks for the o-reduction matmul
            mask8 = consts.tile([H + 1, BL * BL], F32R, tag="mask8")
            nc.sync.dma_start(out=mask8, in_=mask8d.ap())

            eps_sb = consts.tile([128, 1], F32, tag="eps")
            nc.vector.memset(eps_sb, EPS)

            if flags["bk"]:
                bk_sb = consts.tile([128, DT], F32, tag="bk")
                for i in range(DT):
                    nc.sync.dma_start(
                        out=bk_sb[:, i : i + 1], in_=bkt.ap()[i : i + 1, :]
                    )
            ones1 = None
            if need_ones:
                ones1 = consts.tile([1, 128], F32R, tag="ones1")
                nc.sync.dma_start(out=ones1, in_=onesd.ap())
            if flags["bq"]:
                bq_sb = consts.tile([1, D], F32R, tag="bq")
                nc.sync.dma_start(out=bq_sb, in_=bqr.ap())
            if flags["bv"]:
                bv_sb = consts.tile([1, D], F32R, tag="bv")
                nc.sync.dma_start(out=bv_sb, in_=bvr.ap())
            if flags["bp"]:
                bp_sb = consts.tile([1, D], F32R, tag="bp")
                nc.sync.dma_start(out=bp_sb, in_=bpr.ap())
            if flags["b1"]:
                b1_sb = consts.tile([G, HG], F32R, tag="b1")
                nc.sync.dma_start(out=b1_sb, in_=b1r.ap())
            if flags["b2"]:
                b2_sb = consts.tile([G, CG], F32R, tag="b2")
                nc.sync.dma_start(out=b2_sb, in_=b2r.ap())

            # --- persistent accumulator for per-batch o vectors ---
            O_ps = psB.tile([BL, D], F32, tag="ps")

            cp = [0]  # copy-engine rotation counter

            def copy(out, in_, use_act=None):
                """psum->sbuf (or sbuf->sbuf) copy, alternating DVE/ACT."""
                if use_act is None:
                    use_act = cp[0] % 2 == 1
                    cp[0] += 1
                if use_act:
                    nc.scalar.copy(out=out, in_=in_)
                else:
                    nc.vector.tensor_copy(out=out, in_=in_)

            # ================= per-batch pipeline =================
            for b in range(BL):
                nT = big.tile([128, DT * M2], F32R, tag="nT")
                KT = big.tile([128, DT * M2], F32R, tag="KT")
                V = big.tile([128, MT * D], F32R, tag="V")

                # ---- LayerNorm1 (pure normalize; gamma/beta folded into W) ----
                for t in range(MT):
                    rows = 128 if t < MT - 1 else ML2
                    x_t = xpool.tile([128, D], F32, tag="x_t")
                    if t < MT - 1:
                        nc.sync.dma_start(
                            out=x_t[:rows, :], in_=u3[b, t * 128 : t * 128 + rows, :]
                        )
                    else:
                        nc.sync.dma_start(
                            out=x_t[:M_LAST, :], in_=u3[b, t * 128 : M, :]
                        )
                        # duplicate a real row into the pad slot (keeps LN finite)
                        nc.sync.dma_start(
                            out=x_t[M_LAST:ML2, :], in_=u3[b, M - 1 : M, :]
                        )
                    stats = workA.tile([128, 3, 6], F32, tag="stats")
                    for sg in range(3):
                        nc.vector.bn_stats(
                            out=stats[:rows, sg, :],
                            in_=x_t[:rows, sg * 256 : (sg + 1) * 256],
                        )
                    mv = workA.tile([128, 2], F32, tag="mv")
                    nc.vector.bn_aggr(out=mv[:rows, :], in_=stats[:rows, :, :])
                    rstd = workA.tile([128, 1], F32, tag="rstd")
                    nc.scalar.activation(
                        out=rstd[:rows],
                        in_=mv[:rows, 1:2],
                        func=AF.Sqrt,
                        bias=eps_sb[:rows],
                    )
                    nc.vector.reciprocal(out=rstd[:rows], in_=rstd[:rows])
                    # in-place normalize
                    nc.vector.tensor_scalar(
                        out=x_t[:rows, :],
                        in0=x_t[:rows, :],
                        scalar1=mv[:rows, 0:1],
                        scalar2=rstd[:rows],
                        op0=ALU.subtract,
                        op1=ALU.mult,
                    )
                    # transpose into nT
                    for d in range(DT):
                        tp = psT.tile([128, 128], F32, tag="tp")
                        nc.tensor.transpose(
                            out=tp[:, :rows],
                            in_=x_t[:rows, d * 128 : (d + 1) * 128],
                            identity=id_sb[:rows, :rows],
                        )
                        copy(
                            nT[:, d * M2 + t * 128 : d * M2 + t * 128 + rows],
                            tp[:, :rows],
                        )

                if PHASE < 2:
                    continue
                # ---- K^T = Wk^T @ n^T   [d, m] ----
                for d in range(DT):
                    kt_ps = psB.tile([128, M2], F32, tag="ps")
                    for c0, cn in ((0, 512), (512, M2 - 512)):
                        for di in range(DT):
                            nc.tensor.matmul(
                                out=kt_ps[:, c0 : c0 + cn],
                                lhsT=r(wk_sb[:, di * D + d * 128 : di * D + (d + 1) * 128]),
                                rhs=r(nT[:, di * M2 + c0 : di * M2 + c0 + cn]),
                                start=(di == 0),
                                stop=(di == DT - 1),
                            )
                    if flags["bk"]:
                        nc.scalar.activation(
                            out=KT[:, d * M2 : (d + 1) * M2],
                            in_=kt_ps[:, :M2],
                            func=AF.Identity,
                            bias=bk_sb[:, d : d + 1],
                        )
                    else:
                        copy(KT[:, d * M2 : (d + 1) * M2], kt_ps[:, :M2])

                # ---- V natural  [m, d] ----
                for t in range(MT):
                    rows = 128 if t < MT - 1 else M_LAST
                    v_ps = psB.tile([128, D], F32, tag="ps")
                    for c0, cn in ((0, 512), (512, 256)):
                        for di in range(DT):
                            nc.tensor.matmul(
                                out=v_ps[:rows, c0 : c0 + cn],
                                lhsT=r(nT[:, di * M2 + t * 128 : di * M2 + t * 128 + rows]),
                                rhs=r(wv_sb[:, di * D + c0 : di * D + c0 + cn]),
                                start=(di == 0),
                                stop=(di == DT - 1 and not flags["bv"]),
                            )
                        if flags["bv"]:
                            nc.tensor.matmul(
                                out=v_ps[:rows, c0 : c0 + cn],
                                lhsT=r(ones1[:, :rows]),
                                rhs=r(bv_sb[:, c0 : c0 + cn]),
                                start=False,
                                stop=True,
                            )
                    copy(V[:rows, t * D : (t + 1) * D], v_ps[:rows, :])

                if PHASE < 3:
                    continue
                # ---- q for the cls row (row 0 of this batch) ----
                q_ps = psB.tile([1, D], F32, tag="ps")
                for c0, cn in ((0, 512), (512, 256)):
                    for di in range(DT):
                        nc.tensor.matmul(
                            out=q_ps[:, c0 : c0 + cn],
                            lhsT=r(nT[:, di * M2 : di * M2 + 1]),
                            rhs=r(wq_sb[:, di * D + c0 : di * D + c0 + cn]),
                            start=(di == 0),
                            stop=(di == DT - 1 and not flags["bq"]),
                        )
                    if flags["bq"]:
                        nc.tensor.matmul(
                            out=q_ps[:, c0 : c0 + cn],
                            lhsT=r(ones1[:, :1]),
                            rhs=r(bq_sb[:, c0 : c0 + cn]),
                            start=False,
                            stop=True,
                        )
                q_sb = workB.tile([1, D], F32, tag="q_sb")
                copy(q_sb, q_ps[:1, :])
                # transpose q -> qT [d, 1] pieces
                qT = workA.tile([128, DT], F32, tag="qT")
                for d in range(DT):
                    tp = psT.tile([128, 128], F32, tag="tp")
                    nc.tensor.transpose(
                        out=tp[:, :1],
                        in_=q_sb[:1, d * 128 : (d + 1) * 128],
                        identity=id_sb[:1, :1],
                    )
                    copy(qT[:, d : d + 1], tp[:, :1])

                # ---- Qmix[d, g] = qT[d] * W1exp[d, g]  (SCALE+I+w1 folded) ----
                qmix = workA.tile([128, DT * H], F32R, tag="qmix")
                for d in range(DT):
                    nc.vector.tensor_scalar_mul(
                        out=qmix[:, d * H : (d + 1) * H],
                        in0=w1_sb[:, d * H : (d + 1) * H],
                        scalar1=qT[:, d : d + 1],
                    )

                # ---- attn2 = Qmix^T @ K^T   [g=32, m=785] ----
                at_ps = psB.tile([H, M2], F32, tag="ps")
                for c0, cn in ((0, 512), (512, M2 - 512)):
                    for di in range(DT):
                        nc.tensor.matmul(
                            out=at_ps[:, c0 : c0 + cn],
                            lhsT=r(qmix[:, di * H : (di + 1) * H]),
                            rhs=r(KT[:, di * M2 + c0 : di * M2 + c0 + cn]),
                            start=(di == 0),
                            stop=(di == DT - 1),
                        )

                # ---- softmax over m (w1_b shift-invariant => dropped) ----
                negmax = workA.tile([H, 1], F32, tag="negmax")
                nc.vector.reduce_max(
                    out=negmax, in_=at_ps[:H, :M], axis=AX.X, negate=True
                )
                asm = workA.tile([H + 1, M2], F32, tag="asm")
                sums = workA.tile([H, 1], F32, tag="sums")
                nc.scalar.activation(
                    out=asm[:H, :M],
                    in_=at_ps[:H, :M],
                    func=AF.Exp,
                    bias=negmax,
                    accum_out=sums,
                )
                rinv = workA.tile([H, 1], F32, tag="rinv")
                nc.vector.reciprocal(out=rinv, in_=sums)
                nc.vector.tensor_scalar_mul(
                    out=asm[:H, :M], in0=asm[:H, :M], scalar1=rinv
                )
                # ones row (gives colsum(V) -> w2_b term)
                nc.vector.memset(asm[H : H + 1, :], 1.0)
                # zero the pad column so transposed garbage can't leak
                nc.vector.memset(asm[:, M:M2], 0.0)

                # ---- attn^T via PE transposes  [m, 33] ----
                asT = workA.tile([128, MT * (H + 1)], F32R, tag="asT")
                for t in range(MT):
                    rows = 128 if t < MT - 1 else ML2
                    tp = psT.tile([128, 128], F32, tag="tp")
                    nc.tensor.transpose(
                        out=tp[:rows, : H + 1],
                        in_=asm[:, t * 128 : t * 128 + rows],
                        identity=id_sb[: H + 1, : H + 1],
                    )
                    copy(
                        asT[:rows, t * (H + 1) : (t + 1) * (H + 1)],
                        tp[:rows, : H + 1],
                    )

                # ---- OT[h, d] = attn_sm^T^T @ V  [33, 768] ----
                ot_ps = psB.tile([H + 1, D], F32, tag="ps")
                for c0, cn in ((0, 512), (512, 256)):
                    for t in range(MT):
                        rows = 128 if t < MT - 1 else M_LAST
                        nc.tensor.matmul(
                            out=ot_ps[:, c0 : c0 + cn],
                            lhsT=r(asT[:rows, t * (H + 1) : (t + 1) * (H + 1)]),
                            rhs=r(V[:rows, t * D + c0 : t * D + c0 + cn]),
                            start=(t == 0),
                            stop=(t == MT - 1),
                        )

                # ---- mixed = OT * W2exp ; o[b,:] += ones^T @ mixed ----
                mixed = workB.tile([H + 1, D], F32R, tag="mixed")
                nc.vector.tensor_mul(out=mixed, in0=ot_ps[: H + 1, :], in1=w2_sb)
                for c0, cn in ((0, 512), (512, 256)):
                    nc.tensor.matmul(
                        out=O_ps[:, c0 : c0 + cn],
                        lhsT=r(mask8[:, b * BL : (b + 1) * BL]),
                        rhs=r(mixed[:, c0 : c0 + cn]),
                        start=(b == 0),
                        stop=(b == BL - 1),
                    )

            # ================= epilogue: proj + LN2 + grouped MLP =================
            if PHASE < 4:
                out_sb = workB.tile([BL, D], F32, tag="out_sb")
                if PHASE >= 3:
                    copy(out_sb, O_ps[:BL, :])
                else:
                    nc.vector.memset(out_sb, 0.0)
                nc.sync.dma_start(out=xo.ap(), in_=out_sb)
                raise _PhaseDone
            O_sb = workB.tile([BL, D], F32, tag="O_sb")
            copy(O_sb, O_ps[:BL, :])
            OT_sb = workB.tile([128, DT * BL], F32R, tag="OT_sb")
            for d in range(DT):
                tp = psT.tile([128, 128], F32, tag="tp")
                nc.tensor.transpose(
                    out=tp[:, :BL],
                    in_=O_sb[:, d * 128 : (d + 1) * 128],
                    identity=id_sb[:BL, :BL],
                )
                copy(OT_sb[:, d * BL : (d + 1) * BL], tp[:, :BL])

            xc_ps = psB.tile([BL, D], F32, tag="ps")
            for di in range(DT):
                wp_t = wppool.tile([128, D], F32R, tag="wp")
                nc.sync.dma_start(
                    out=wp_t, in_=wp.ap()[di * 128 : (di + 1) * 128, :]
                )
                for c0, cn in ((0, 512), (512, 256)):
                    nc.tensor.matmul(
                        out=xc_ps[:, c0 : c0 + cn],
                        lhsT=r(OT_sb[:, di * BL : (di + 1) * BL]),
                        rhs=r(wp_t[:, c0 : c0 + cn]),
                        start=(di == 0),
                        stop=(di == DT - 1 and not flags["bp"]),
                    )
            if flags["bp"]:
                for c0, cn in ((0, 512), (512, 256)):
                    nc.tensor.matmul(
                        out=xc_ps[:, c0 : c0 + cn],
                        lhsT=r(ones1[:, :BL]),
                        rhs=r(bp_sb[:, c0 : c0 + cn]),
                        start=False,
                        stop=True,
                    )

            # residual: x_cls2 = x_cls_raw + proj_out
            xraw = workB.tile([BL, D], F32, tag="xraw")
            nc.sync.dma_start(out=xraw, in_=u3[:, 0, :])
            xc2 = workB.tile([BL, D], F32, tag="xc2")
            nc.vector.tensor_add(out=xc2, in0=xc_ps[:BL, :], in1=xraw)

            # ---- LN2 (gamma/beta folded into fc1) ----
            stats2 = workB.tile([BL, 3, 6], F32, tag="stats2")
            for sg in range(3):
                nc.vector.bn_stats(
                    out=stats2[:, sg, :], in_=xc2[:, sg * 256 : (sg + 1) * 256]
                )
            mv2 = workB.tile([BL, 2], F32, tag="mv2")
            nc.vector.bn_aggr(out=mv2, in_=stats2)
            rstd2 = workB.tile([BL, 1], F32, tag="rstd2")
            nc.scalar.activation(
                out=rstd2, in_=mv2[:, 1:2], func=AF.Sqrt, bias=eps_sb[:BL]
            )
            nc.vector.reciprocal(out=rstd2, in_=rstd2)
            y_sb = workB.tile([BL, D], F32, tag="y_sb")
            nc.vector.tensor_scalar(
                out=y_sb,
                in0=xc2,
                scalar1=mv2[:, 0:1],
                scalar2=rstd2,
                op0=ALU.subtract,
                op1=ALU.mult,
            )

            # ---- yT per group  [c, b] ----
            yT = workB.tile([128, DT * BL], F32R, tag="yT")
            for d in range(DT):
                tp = psT.tile([128, 128], F32, tag="tp")
                nc.tensor.transpose(
                    out=tp[:, :BL],
                    in_=y_sb[:, d * 128 : (d + 1) * 128],
                    identity=id_sb[:BL, :BL],
                )
                copy(yT[:, d * BL : (d + 1) * BL], tp[:, :BL])

            # ---- fc1 per group: h[b, o] (o in 0..1536), then exact GELU ----
            hg_sb = workB.tile([BL, HID], F32, tag="hg_sb")
            CT1 = CG // 128  # 3 contraction tiles per group
            for g in range(G):
                for half in range(2):
                    base = half * (HG // 2)
                    fcw_tiles = []
                    for ct in range(CT1):
                        fcw = fc1pool.tile([128, HG // 2], F32R, tag="fc1w")
                        nc.sync.dma_start(
                            out=fcw,
                            in_=fc1t.ap()[g, ct * 128 : (ct + 1) * 128, base : base + HG // 2],
                        )
                        fcw_tiles.append(fcw)
                    h_ps = psB.tile([BL, HG // 2], F32, tag="ps")
                    for c0, cn in ((0, 512), (512, 256)):
                        for ct in range(CT1):
                            nc.tensor.matmul(
                                out=h_ps[:, c0 : c0 + cn],
                                lhsT=r(yT[:, (g * CT1 + ct) * BL : (g * CT1 + ct + 1) * BL]),
                                rhs=r(fcw_tiles[ct][:, c0 : c0 + cn]),
                                start=(ct == 0),
                                stop=(ct == CT1 - 1 and not flags["b1"]),
                            )
                        if flags["b1"]:
                            nc.tensor.matmul(
                                out=h_ps[:, c0 : c0 + cn],
                                lhsT=r(ones1[:, :BL]),
                                rhs=r(b1_sb[g : g + 1, base + c0 : base + c0 + cn]),
                                start=False,
                                stop=True,
                            )
                    nc.scalar.activation(
                        out=hg_sb[:, g * HG + base : g * HG + base + HG // 2],
                        in_=h_ps[:BL, :],
                        func=AF.Gelu,
                    )

            # ---- channel shuffle + transpose: hT[c, b] for each shuffled group ----
            # shuffled group g2 channel c2  <=  h channel 2*c2 + g2
            hg_v = hg_sb.rearrange("p (c two) -> p two c", two=G)
            CT2 = HG // 128  # 12
            hT = workB.tile([128, G * CT2 * BL], F32R, tag="hT")
            for g2 in range(G):
                for ct in range(CT2):
                    tp = psT.tile([128, 128], F32, tag="tp")
                    nc.tensor.transpose(
                        out=tp[:, :BL],
                        in_=hg_v[:, g2, ct * 128 : (ct + 1) * 128],
                        identity=id_sb[:BL, :BL],
                    )
                    copy(
                        hT[:, (g2 * CT2 + ct) * BL : (g2 * CT2 + ct + 1) * BL],
                        tp[:, :BL],
                    )

            # ---- fc2 per group + residual add -> output ----
            out_sb = workB.tile([BL, D], F32, tag="out_sb")
            for g2 in range(G):
                z_ps = psB.tile([BL, CG], F32, tag="ps")
                for ct in range(CT2):
                    fw = fc2pool.tile([128, CG], F32R, tag="fc2w")
                    nc.sync.dma_start(
                        out=fw, in_=fc2t.ap()[g2, ct * 128 : (ct + 1) * 128, :]
                    )
                    nc.tensor.matmul(
                        out=z_ps[:, :],
                        lhsT=r(hT[:, (g2 * CT2 + ct) * BL : (g2 * CT2 + ct + 1) * BL]),
                        rhs=r(fw),
                        start=(ct == 0),
                        stop=(ct == CT2 - 1 and not flags["b2"]),
                    )
                if flags["b2"]:
                    nc.tensor.matmul(
                        out=z_ps[:, :],
                        lhsT=r(ones1[:, :BL]),
                        rhs=r(b2_sb[g2 : g2 + 1, :]),
                        start=False,
                        stop=True,
                    )
                nc.vector.tensor_add(
                    out=out_sb[:, g2 * CG : (g2 + 1) * CG],
                    in0=z_ps[:BL, :],
                    in1=xc2[:, g2 * CG : (g2 + 1) * CG],
                )

            nc.sync.dma_start(out=xo.ap(), in_=out_sb)
        except _PhaseDone:
            pass

    split_excess_waits(nc)
    return nc


_CACHED = {}


def _get_program(flags):
    key = tuple(sorted(flags.items()))
    if key not in _CACHED:
        _CACHED[key] = build_program(flags)
    return _CACHED[key]


def prepare_in_maps(inputs):
    """Host-side folds + per-core sharding. Returns (in_maps, nc)."""
    f32 = np.float32
    x_cls = np.asarray(inputs["x_cls"], f32)
    x_img_np = np.asarray(inputs["x_img"], f32)
    ln1_g = np.asarray(inputs["ln1_g"], f32)
    ln1_b = np.asarray(inputs["ln1_b"], f32)
    Wq = np.asarray(inputs["Wq"], f32); bq = np.asarray(inputs["bq"], f32)
    Wk = np.asarray(inputs["Wk"], f32); bk = np.asarray(inputs["bk"], f32)
    Wv = np.asarray(inputs["Wv"], f32); bv = np.asarray(inputs["bv"], f32)
    w1_W = np.asarray(inputs["w1_W"], f32)
    w2_W = np.asarray(inputs["w2_W"], f32)
    w2_b = np.asarray(inputs["w2_b"], f32)
    proj_W = np.asarray(inputs["proj_W"], f32)
    proj_b = np.asarray(inputs["proj_b"], f32)
    ln2_g = np.asarray(inputs["ln2_g"], f32)
    ln2_b = np.asarray(inputs["ln2_b"], f32)
    fc1_W = np.asarray(inputs["fc1_W"], f32)
    fc1_b = np.asarray(inputs["fc1_b"], f32)
    fc2_W = np.asarray(inputs["fc2_W"], f32)
    fc2_b = np.asarray(inputs["fc2_b"], f32)

    # ---- host-side exact folds ----
    Wq_f = (ln1_g[:, None] * Wq).astype(f32)
    Wk_f = (ln1_g[:, None] * Wk).astype(f32)
    Wv_f = (ln1_g[:, None] * Wv).astype(f32)
    bq_f = (bq + ln1_b @ Wq).astype(f32)
    bk_f = (bk + ln1_b @ Wk).astype(f32)
    bv_f = (bv + ln1_b @ Wv).astype(f32)
    W1e = (np.eye(H, dtype=f32) + w1_W) * SCALE
    W1exp = np.repeat(W1e, HD, axis=0).astype(f32)          # [768, 32]
    W2x = np.concatenate([np.eye(H, dtype=f32) + w2_W, w2_b[None, :]], axis=0)
    W2exp = np.repeat(W2x, HD, axis=1).astype(f32)          # [33, 768]
    fc1T = np.ascontiguousarray(fc1_W.transpose(0, 2, 1))   # [G, 384, 1536]
    fc1T_f = np.ascontiguousarray(fc1T * ln2_g.reshape(G, CG, 1)).astype(f32)
    fc1b_f = (fc1_b + np.einsum("goc,gc->go", fc1_W, ln2_b.reshape(G, CG))).astype(f32)
    fc2T = np.ascontiguousarray(fc2_W.transpose(0, 2, 1)).astype(f32)  # [G, 1536, 384]

    flags = {
        "bq": bool(np.any(bq_f)),
        "bk": bool(np.any(bk_f)),
        "bv": bool(np.any(bv_f)),
        "bp": bool(np.any(proj_b)),
        "b1": bool(np.any(fc1b_f)),
        "b2": bool(np.any(fc2_b)),
    }
    nc = _get_program(flags)

    ident = np.eye(128, dtype=f32)
    mask8 = np.zeros((H + 1, BL * BL), dtype=f32)
    for b in range(BL):
        mask8[:, b * BL + b] = 1.0
    shared = {
        "wq": Wq_f, "wk": Wk_f, "wv": Wv_f, "wp": proj_W,
        "w1e": W1exp, "w2e": W2exp,
        "fc1t": fc1T_f, "fc2t": fc2T, "ident": ident, "mask8d": mask8,
    }
    if any(flags[k] for k in ("bq", "bv", "bp", "b1", "b2")):
        shared["onesd"] = np.ones((1, 128), dtype=f32)
    if flags["bk"]:
        shared["bkt"] = bk_f.reshape(DT, 128)
    if flags["bq"]:
        shared["bqr"] = bq_f.reshape(1, D)
    if flags["bv"]:
        shared["bvr"] = bv_f.reshape(1, D)
    if flags["bp"]:
        shared["bpr"] = proj_b.reshape(1, D)
    if flags["b1"]:
        shared["b1r"] = fc1b_f
    if flags["b2"]:
        shared["b2r"] = fc2_b

    xc3 = x_cls.reshape(B, 1, D)
    in_maps = []
    for c in range(N_CORES):
        sl = slice(c * BL, (c + 1) * BL)
        u_c = np.concatenate([xc3[sl], x_img_np[sl]], axis=1).reshape(BL * M, D)
        in_maps.append({"u": np.ascontiguousarray(u_c), **shared})
    return in_maps, nc


def kernel(x_cls, x_img, **rest):
    inputs = {"x_cls": x_cls, "x_img": x_img, **rest}
    in_maps, nc = prepare_in_maps(inputs)
    x_img_np = np.asarray(x_img, np.float32)
    res = run_bass_kernel_spmd(nc, in_maps, core_ids=list(range(N_CORES)))
    x_cls_out = np.concatenate(
        [res.results[c]["xo"].reshape(BL, 1, D) for c in range(N_CORES)], axis=0
    )
    return (x_cls_out, x_img_np)
